# revision 12
# baseline (speedup 1.0000x reference)
"""Trainium2 Bass kernel for nn_AttnResBlock (B=16, C=512, A=64, L=1024).

Data-parallel over batch: 8 cores x 2 batches each, weights replicated.
BatchNorm (training mode, stats over (B, L)) needs global batch stats ->
two tiny [128, 8] f32 AllReduces; a same-shape warmup collective at
kernel start absorbs the first-collective setup cost under input DMA.

Precision (rel-err budget 2e-2, measured ~3e-3):
  - x, xT arrive bf16 (halves input DMA); x2 and the output stay f32.
  - attnout + proj matmuls run fp8(e4m3) DoubleRow: 2 contraction
    rows/partition/cycle -> half the matmuls. xTs carries S_A=256 (the
    softmax recip is ~1e-3, below fp8 normal range), wp carries S_W=16;
    1/(S_A*S_W) folds into the x2 epilogue scalar. fp8 noise lands only
    on the attention output, which is small next to the residual.
  - convs + kq + scores stay bf16: fp8 would put ~3% relative noise on
    conv outputs (quantization noise does not average down in
    incoherent sums), blowing the budget.

Schedule:
  - Engine-order stalls sequence the DMA traffic: x/xT(b0) first, conv
    weights issue from the gpsimd queue only after the b0 queries move
    (so they never race the phase-A-critical loads; the old kernel lost
    ~10us to weight traffic queued in front of the BN1 stats DMA).
  - Phase A is software-pipelined across the two local batches: kq(b1)
    runs right after kq(b0) so b1's keys/queries are staged long before
    the PE reaches b1's scores.
  - x is pre-biased into xb = x + bp during the load window, so the x2
    epilogue is one DVE op (psum*s + xb, channel sums via accum_out);
    sums-of-squares ride ACT Square accum_out. BN stats cost no
    dedicated passes.
  - Convs run hc-outer / oc-inner with two PSUM banks (b) per step; the
    last group's epilogue is ~1.5us, and conv2 streams 512-col output
    chunks to HBM as they finish.
"""
import numpy as np

P = 128
B, C, A, L = 16, 512, 64, 1024
NCORES = 8
BL = B // NCORES          # local batches per core
CT = C // P               # 4 channel tiles
LT = L // P               # 8 length tiles
MC = L // 512             # 2 moving chunks
EPS = 1e-5
SM_SCALE = 2.0 / L        # softmax scale: scores/(L/2)
S_W = 16.0                # fp8 scale for wp
S_A = 256.0               # fp8 scale for xTs

_CACHE = {}


def _build():
    import concourse.bass as bass
    import concourse.mybir as mybir
    from concourse import bacc
    from concourse.tile import TileContext

    f32 = mybir.dt.float32
    bf16 = mybir.dt.bfloat16
    fp8 = mybir.dt.float8e4
    AF = mybir.ActivationFunctionType
    ALU = mybir.AluOpType
    DR = mybir.MatmulPerfMode.DoubleRow

    nc = bacc.Bacc(num_devices=NCORES)

    x_ext = nc.declare_dram_parameter("x", [BL, C, L], bf16, isOutput=False)
    xT_ext = nc.declare_dram_parameter("xT", [BL, L, C], bf16, isOutput=False)
    wkq_ext = nc.declare_dram_parameter("wkq", [P, CT * 2 * A], bf16, isOutput=False)
    wp_ext = nc.declare_dram_parameter("wp", [P, CT * C], fp8, isOutput=False)
    w1_ext = nc.declare_dram_parameter("w1", [P, 3 * CT * C], bf16, isOutput=False)
    w2_ext = nc.declare_dram_parameter("w2", [P, 3 * CT * C], bf16, isOutput=False)
    # per-channel vectors packed [P, CT] each: bp b1 b2 g1 be1 g2 be2, then bkq
    pvec_ext = nc.declare_dram_parameter("pvec", [P, 7 * CT + 1], f32, isOutput=False)
    out_ext = nc.declare_dram_parameter("out", [BL, C, L], f32, isOutput=True)

    cc0_in = nc.dram_tensor("cc0_in", [P, 2 * CT], f32)
    cc0_out = nc.dram_tensor("cc0_out", [P, 2 * CT], f32, addr_space="Shared")
    cc1_in = nc.dram_tensor("cc1_in", [P, 2 * CT], f32)
    cc1_out = nc.dram_tensor("cc1_out", [P, 2 * CT], f32, addr_space="Shared")
    cc2_in = nc.dram_tensor("cc2_in", [P, 2 * CT], f32)
    cc2_out = nc.dram_tensor("cc2_out", [P, 2 * CT], f32, addr_space="Shared")

    rg = [list(range(NCORES))]

    with TileContext(nc) as tc:
        with tc.tile_pool(name="pers", bufs=1) as pers, \
             tc.tile_pool(name="small", bufs=1) as small, \
             tc.tile_pool(name="ostage", bufs=4) as ostage, \
             tc.tile_pool(name="phA", bufs=2) as pab, \
             tc.tile_pool(name="psum", bufs=8, space="PSUM") as psum:

            # ---- kernel-lifetime tiles ----
            x2_sb = pers.tile([P, BL, CT, L], f32)
            x_sb = pers.tile([P, BL, CT, L], bf16)
            xb_sb = pers.tile([P, BL, CT, L], bf16)     # x + bp (residual side)
            wkq_sb = pers.tile([P, CT, 2 * A], bf16)
            wp_sb = pers.tile([P, CT, C], fp8)
            w1_sb = pers.tile([P, 3 * CT, C], bf16)
            w2_sb = pers.tile([P, 3 * CT, C], bf16)
            h_sb = pers.tile([P, BL, CT, L + 2], bf16)
            h2_sb = pers.tile([P, BL, CT, L], bf16)
            keys_sb = pers.tile([P, BL, L], bf16)       # keys 0:A, queries A:2A
            queries_sb = pers.tile([P, BL, L], bf16)    # queries at base 0

            # warmup collective: absorbs the first-collective setup cost
            nc.gpsimd.collective_compute(
                "AllReduce", mybir.AluOpType.add, replica_groups=rg,
                ins=[cc0_in[:].opt()], outs=[cc0_out[:].opt()])

            pvec_sb = small.tile([P, 7 * CT + 1], f32, tag="pvec")
            nc.gpsimd.dma_start(out=pvec_sb[:], in_=pvec_ext[:])
            bp_sb = pvec_sb[:, 0 * CT:1 * CT]
            b1_sb = pvec_sb[:, 1 * CT:2 * CT]
            b2_sb = pvec_sb[:, 2 * CT:3 * CT]
            g1_sb = pvec_sb[:, 3 * CT:4 * CT]
            be1_sb = pvec_sb[:, 4 * CT:5 * CT]
            g2_sb = pvec_sb[:, 5 * CT:6 * CT]
            be2_sb = pvec_sb[:, 6 * CT:7 * CT]
            bkq_sb = pvec_sb[:, 7 * CT:7 * CT + 1]   # [bk; bq]

            # phase-A-critical loads on the sync queue: wkq then x
            nc.sync.dma_start(out=wkq_sb[:],
                              in_=wkq_ext[:].rearrange("p (ct a) -> p ct a", ct=CT))
            for b in range(BL):
                for mc in range(MC):
                    for ct in range(CT):
                        nc.sync.dma_start(
                            out=x_sb[:, b, ct, mc * 512:(mc + 1) * 512],
                            in_=x_ext[b, ct * P:(ct + 1) * P, mc * 512:(mc + 1) * 512])
            # xT(b0) early on gpsimd; xT(b1) + conv weights issue later from
            # the same queue, after it stalls on the b0 queries move
            xT_tiles = [pab.tile([P, LT, C], bf16, tag="xT", name=f"xT{_b}")
                        for _b in range(BL)]
            nc.gpsimd.dma_start(out=xT_tiles[0][:],
                                in_=xT_ext[0].rearrange("(lc p) c -> p lc c", p=P))

            ccin1_sb = small.tile([P, 2 * CT], f32, tag="ccin1")
            ccout1_sb = small.tile([P, 2 * CT], f32, tag="ccout1")
            ccin2_sb = small.tile([P, 2 * CT], f32, tag="ccin2")
            ccout2_sb = small.tile([P, 2 * CT], f32, tag="ccout2")
            # per-chunk stat accumulators [P, ct, 2*b+chunk]
            m1a = small.tile([P, CT, 2 * BL], f32, tag="m1a")   # sum(x2)
            m2a = small.tile([P, CT, 2 * BL], f32, tag="m2a")   # sum(x2^2)
            n1a = small.tile([P, CT, 2 * BL], f32, tag="n1a")   # sum(h2)
            n2a = small.tile([P, CT, 2 * BL], f32, tag="n2a")   # sum(h2^2)
            scale1 = small.tile([P, CT], f32, tag="scale1")
            bias1 = small.tile([P, CT], f32, tag="bias1")
            scale2 = small.tile([P, CT], f32, tag="scale2")
            bias2 = small.tile([P, CT], f32, tag="bias2")
            eps_sb = small.tile([P, 1], f32, tag="eps")
            nc.vector.memset(eps_sb[:], EPS)

            # conv padding zeros (columns 0 and L+1 of every (b, ct) row)
            nc.vector.memset(h_sb[:, :, :, 0], 0.0)
            nc.vector.memset(h_sb[:, :, :, L + 1], 0.0)

            # pre-warm ACT function tables (only 4 funcs used all kernel)
            warm = small.tile([P, 1], f32, tag="warm")
            for fn in (AF.Square, AF.Sqrt, AF.Relu, AF.Exp):
                nc.scalar.activation(out=warm[:], in_=eps_sb[:], func=fn)

            def stats_allreduce(ccin_dram, ccout_dram, ccin_sb, ccred_sb):
                nc.gpsimd.dma_start(out=ccin_dram[:], in_=ccin_sb[:])
                nc.gpsimd.collective_compute(
                    "AllReduce", mybir.AluOpType.add, replica_groups=rg,
                    ins=[ccin_dram[:].opt()], outs=[ccout_dram[:].opt()])
                nc.gpsimd.dma_start(out=ccred_sb[:], in_=ccout_dram[:])

            def pack_stats(msum, sqsum, ccin_sb):
                nc.vector.tensor_reduce(out=ccin_sb[:, 0:CT], in_=msum[:],
                                        axis=mybir.AxisListType.X, op=ALU.add)
                nc.vector.tensor_reduce(out=ccin_sb[:, CT:2 * CT], in_=sqsum[:],
                                        axis=mybir.AxisListType.X, op=ALU.add)

            def bn_post(ccout_sb, g_sb, be_sb, scale_t, bias_t, tag):
                mgx = small.tile([P, 2 * CT], f32, tag=tag + "mgx")
                nc.vector.tensor_scalar_mul(out=mgx[:], in0=ccout_sb[:],
                                            scalar1=1.0 / (B * L))
                mg = mgx[:, 0:CT]
                ex2 = mgx[:, CT:2 * CT]
                nvar = small.tile([P, CT], f32, tag=tag + "nv")
                # nvar = mean^2 - E[x^2] = -var
                nc.vector.tensor_tensor(out=nvar[:], in0=mg, in1=mg, op=ALU.mult)
                nc.vector.tensor_tensor(out=nvar[:], in0=nvar[:], in1=ex2, op=ALU.subtract)
                sd = small.tile([P, CT], f32, tag=tag + "sd")
                nc.scalar.activation(out=sd[:], in_=nvar[:], func=AF.Sqrt,
                                     scale=-1.0, bias=eps_sb[:])
                rstd = small.tile([P, CT], f32, tag=tag + "rstd")
                nc.vector.reciprocal(out=rstd[:], in_=sd[:])
                nc.vector.tensor_tensor(out=scale_t[:], in0=rstd[:], in1=g_sb[:], op=ALU.mult)
                tmp = small.tile([P, CT], f32, tag=tag + "tmp")
                nc.vector.tensor_tensor(out=tmp[:], in0=mg, in1=scale_t[:], op=ALU.mult)
                nc.vector.tensor_tensor(out=bias_t[:], in0=be_sb[:], in1=tmp[:], op=ALU.subtract)

            # ---------------- Phase A: attention ----------------
            # kq for BOTH batches first (pipelines b1's staging under b0)
            for b in range(BL):
                for mc in range(MC):
                    ms = slice(mc * 512, (mc + 1) * 512)
                    kps = psum.tile([P, 512], f32, tag="ps")
                    for ct in range(CT):
                        nc.tensor.matmul(
                            out=kps[:],
                            lhsT=wkq_sb[:, ct, :],
                            rhs=x_sb[:, b, ct, ms],
                            start=(ct == 0), stop=(ct == CT - 1))
                    # rows 0:64 keys+bk, 64:128 queries+bq (one DVE op)
                    nc.vector.tensor_scalar_add(out=keys_sb[:, b, ms],
                                                in0=kps[:], scalar1=bkq_sb)
                # queries to partition base 0 (SBUF->SBUF remap)
                nc.gpsimd.dma_start(out=queries_sb[0:A, b, :],
                                    in_=keys_sb[A:2 * A, b, :])
                if b == 0:
                    # gpsimd reaches here only after the b0 queries move:
                    # xT(b1) + conv weights now load behind phase-A traffic
                    nc.gpsimd.dma_start(out=xT_tiles[1][:],
                                        in_=xT_ext[1].rearrange("(lc p) c -> p lc c", p=P))
                    nc.gpsimd.dma_start(out=w1_sb[:],
                                        in_=w1_ext[:].rearrange("p (kc c) -> p kc c", c=C))
                    nc.gpsimd.dma_start(out=w2_sb[:],
                                        in_=w2_ext[:].rearrange("p (kc c) -> p kc c", c=C))
                    nc.gpsimd.dma_start(out=wp_sb[:],
                                        in_=wp_ext[:].rearrange("p (ct o) -> p ct o", ct=CT))

            # x + bp staged once for the x2 epilogue (DVE, hidden under kq)
            for b in range(BL):
                for ct in range(CT):
                    nc.gpsimd.tensor_scalar_add(out=xb_sb[:, b, ct, :],
                                                in0=x_sb[:, b, ct, :],
                                                scalar1=bp_sb[:, ct:ct + 1])

            for b in range(BL):
                xT_sb = xT_tiles[b]
                e_sb = pab.tile([P, LT, L], fp8, tag="e")
                rsp = pab.tile([P, LT, MC], f32, tag="rsp")
                rcp = pab.tile([P, LT], f32, tag="rcp")
                xTs = pab.tile([P, LT, C], fp8, tag="xTs")
                for lc in range(LT):
                    for mc in range(MC):
                        sps = psum.tile([P, 512], f32, tag="ps")
                        nc.tensor.matmul(
                            out=sps[:],
                            lhsT=keys_sb[0:A, b, lc * P:(lc + 1) * P],
                            rhs=queries_sb[0:A, b, mc * 512:(mc + 1) * 512],
                            start=True, stop=True)
                        # exp -> fp8, free row sums via the ACT accumulator
                        nc.scalar.activation(
                            out=e_sb[:, lc, mc * 512:(mc + 1) * 512],
                            in_=sps[:], func=AF.Exp, scale=SM_SCALE,
                            accum_out=rsp[:, lc, mc:mc + 1])
                    nc.vector.scalar_tensor_tensor(
                        out=rcp[:, lc:lc + 1], in0=rsp[:, lc, 0:1],
                        scalar=1.0, in1=rsp[:, lc, 1:2],
                        op0=ALU.mult, op1=ALU.add)
                    nc.vector.reciprocal(out=rcp[:, lc:lc + 1],
                                         in_=rcp[:, lc:lc + 1])
                    # xTs[l, c] = xT[l, c] * (S_A / rowsum[l]) -> fp8
                    nc.vector.tensor_scalar(out=xTs[:, lc, :],
                                            in0=xT_sb[:, lc, :],
                                            scalar1=rcp[:, lc:lc + 1],
                                            scalar2=S_A,
                                            op0=ALU.mult, op1=ALU.mult)

                ao_sb = pab.tile([P, CT, L], fp8, tag="ao")
                for cc in range(CT):
                    for mc in range(MC):
                        ms = slice(mc * 512, (mc + 1) * 512)
                        aps = psum.tile([P, 512], f32, tag="ps")
                        for lcp in range(LT // 2):
                            nc.tensor.matmul(
                                out=aps[:],
                                lhsT=xTs[:, 2 * lcp:2 * lcp + 2, cc * P:(cc + 1) * P],
                                rhs=e_sb[:, 2 * lcp:2 * lcp + 2, ms],
                                start=(lcp == 0), stop=(lcp == LT // 2 - 1),
                                perf_mode=DR)
                        nc.scalar.activation(out=ao_sb[:, cc, ms],
                                             in_=aps[:], func=AF.Copy)

                for oc in range(CT):
                    for mc in range(MC):
                        ms = slice(mc * 512, (mc + 1) * 512)
                        pps = psum.tile([P, 512], f32, tag="ps")
                        for cp in range(CT // 2):
                            nc.tensor.matmul(
                                out=pps[:],
                                lhsT=wp_sb[:, 2 * cp:2 * cp + 2, oc * P:(oc + 1) * P],
                                rhs=ao_sb[:, 2 * cp:2 * cp + 2, ms],
                                start=(cp == 0), stop=(cp == CT // 2 - 1),
                                perf_mode=DR)
                        # x2 = psum/(S_W*S_A) + (x + bp); channel sums free
                        nc.vector.scalar_tensor_tensor(
                            out=x2_sb[:, b, oc, ms], in0=pps[:],
                            scalar=1.0 / (S_W * S_A), in1=xb_sb[:, b, oc, ms],
                            op0=ALU.mult, op1=ALU.add,
                            accum_out=m1a[:, oc, 2 * b + mc:2 * b + mc + 1])
                        sqs = ostage.tile([P, 512], f32, tag="sqs")
                        nc.vector.scalar_tensor_tensor(
                            out=sqs[:], in0=x2_sb[:, b, oc, ms],
                            scalar=1.0, in1=x2_sb[:, b, oc, ms],
                            op0=ALU.mult, op1=ALU.mult,
                            accum_out=m2a[:, oc, 2 * b + mc:2 * b + mc + 1])

            # ---------------- BN1 + conv1 ----------------
            pack_stats(m1a, m2a, ccin1_sb)
            stats_allreduce(cc1_in, cc1_out, ccin1_sb, ccout1_sb)
            for fn in (AF.Sqrt, AF.Relu):   # re-warm while the mesh runs
                nc.scalar.activation(out=warm[:], in_=eps_sb[:], func=fn)
            bn_post(ccout1_sb, g1_sb, be1_sb, scale1, bias1, "p1")

            # h = relu(bn1(x2)), padded; b-major so conv can start after 2 ops
            for b in range(BL):
                for ct in range(CT):
                    nc.scalar.activation(out=h_sb[:, b, ct, 1:L + 1],
                                         in_=x2_sb[:, b, ct, :], func=AF.Relu,
                                         scale=scale1[:, ct:ct + 1],
                                         bias=bias1[:, ct:ct + 1])

            # conv1: hc-outer, oc-inner, 2 psum banks (b)
            for hc in range(MC):
                for oc in range(CT):
                    cps = [psum.tile([P, 512], f32, tag="ps", name=f"c1ps{hc}_{oc}_{_j}")
                           for _j in range(BL)]
                    for ct in range(CT):
                        for k in range(3):
                            w_ap = w1_sb[:, k * CT + ct, oc * P:(oc + 1) * P]
                            for b in range(BL):
                                nc.tensor.matmul(
                                    out=cps[b][:], lhsT=w_ap,
                                    rhs=h_sb[:, b, ct, hc * 512 + k:hc * 512 + k + 512],
                                    start=(ct == 0 and k == 0),
                                    stop=(ct == CT - 1 and k == 2))
                    for b in range(BL):
                        hs = slice(hc * 512, (hc + 1) * 512)
                        nc.vector.tensor_scalar(
                            out=h2_sb[:, b, oc, hs], in0=cps[b][:],
                            scalar1=b1_sb[:, oc:oc + 1], scalar2=0.0,
                            op0=ALU.add, op1=ALU.add,
                            accum_out=n1a[:, oc, 2 * b + hc:2 * b + hc + 1])
                        sqs = ostage.tile([P, 512], f32, tag="sqs")
                        nc.scalar.activation(
                            out=sqs[:], in_=h2_sb[:, b, oc, hs], func=AF.Square,
                            accum_out=n2a[:, oc, 2 * b + hc:2 * b + hc + 1])

            # ---------------- BN2 + conv2 ----------------
            pack_stats(n1a, n2a, ccin2_sb)
            stats_allreduce(cc2_in, cc2_out, ccin2_sb, ccout2_sb)
            for fn in (AF.Sqrt, AF.Relu):   # re-warm while the mesh runs
                nc.scalar.activation(out=warm[:], in_=eps_sb[:], func=fn)
            bn_post(ccout2_sb, g2_sb, be2_sb, scale2, bias2, "p2")

            # h3 = relu(bn2(h2)) overwrites h_sb in place (pad zeros kept)
            for b in range(BL):
                for ct in range(CT):
                    nc.scalar.activation(out=h_sb[:, b, ct, 1:L + 1],
                                         in_=h2_sb[:, b, ct, :], func=AF.Relu,
                                         scale=scale2[:, ct:ct + 1],
                                         bias=bias2[:, ct:ct + 1])

            # conv2 + b2 + residual -> out, streaming 512-col chunks to HBM
            for hc in range(MC):
                for oc in range(CT):
                    cps = [psum.tile([P, 512], f32, tag="ps", name=f"c2ps{hc}_{oc}_{_j}")
                           for _j in range(BL)]
                    for ct in range(CT):
                        for k in range(3):
                            w_ap = w2_sb[:, k * CT + ct, oc * P:(oc + 1) * P]
                            for b in range(BL):
                                nc.tensor.matmul(
                                    out=cps[b][:], lhsT=w_ap,
                                    rhs=h_sb[:, b, ct, hc * 512 + k:hc * 512 + k + 512],
                                    start=(ct == 0 and k == 0),
                                    stop=(ct == CT - 1 and k == 2))
                    for b in range(BL):
                        hs = slice(hc * 512, (hc + 1) * 512)
                        og = ostage.tile([P, 512], f32, tag="og")
                        nc.vector.scalar_tensor_tensor(
                            out=og[:], in0=cps[b][:],
                            scalar=b2_sb[:, oc:oc + 1],
                            in1=x2_sb[:, b, oc, hs],
                            op0=ALU.add, op1=ALU.add)
                        nc.sync.dma_start(
                            out=out_ext[b, oc * P:(oc + 1) * P, hs], in_=og[:])

    nc.compile()
    return nc


def _get_nc():
    if "nc" not in _CACHE:
        _CACHE["nc"] = _build()
    return _CACHE["nc"]


def _prep_in_maps(inputs):
    import ml_dtypes
    f = np.float32
    bf = ml_dtypes.bfloat16
    f8 = ml_dtypes.float8_e4m3
    x = np.ascontiguousarray(inputs["x"], dtype=f)

    def vec_pct(v):
        # (C,) -> [P, CT] with channel c = ct*P + p at [p, ct]
        return np.asarray(v, dtype=f).reshape(CT, P).T

    pvec = np.concatenate(
        [vec_pct(inputs["bp"]), vec_pct(inputs["b1"]), vec_pct(inputs["b2"]),
         vec_pct(inputs["g1"]), vec_pct(inputs["be1"]),
         vec_pct(inputs["g2"]), vec_pct(inputs["be2"]),
         np.concatenate([inputs["bk"], inputs["bq"]]).reshape(P, 1).astype(f)],
        axis=1)

    def swiz2(w):  # [C, X] -> [P, CT*X] partition-major
        X = w.shape[1]
        return np.ascontiguousarray(
            w.reshape(CT, P, X).transpose(1, 0, 2).reshape(P, CT * X))

    def swiz3(w):  # [3, C, C] (k, i, o) -> [P, 3*CT*C] with cols (k*CT+ct)*C+o
        return np.ascontiguousarray(
            w.reshape(3, CT, P, C).transpose(2, 0, 1, 3).reshape(P, 3 * CT * C))

    shared = {
        "wkq": swiz2(np.concatenate([inputs["Wk"].T, inputs["Wq"].T], axis=1).astype(bf)),
        "wp": swiz2((inputs["Wp"].T * S_W).astype(f8)),
        "w1": swiz3(np.transpose(inputs["W1"], (2, 1, 0)).astype(bf)),
        "w2": swiz3(np.transpose(inputs["W2"], (2, 1, 0)).astype(bf)),
        "pvec": np.ascontiguousarray(pvec, dtype=f),
    }
    in_maps = []
    for i in range(NCORES):
        xl = np.ascontiguousarray(x[i * BL:(i + 1) * BL])
        xTl = np.ascontiguousarray(np.transpose(xl, (0, 2, 1)).astype(bf))
        m = {"x": xl.astype(bf), "xT": xTl}
        m.update(shared)
        in_maps.append(m)
    return in_maps


def kernel(**inputs) -> np.ndarray:
    from concourse import bass_utils
    nc = _get_nc()
    in_maps = _prep_in_maps(inputs)
    res = bass_utils.run_bass_kernel_spmd(nc, in_maps, list(range(NCORES)))
    return np.concatenate([r["out"] for r in res.results], axis=0)


# revision 13
# speedup vs baseline: 1.4499x; 1.4499x over previous
"""Trainium2 Bass kernel for nn_AttnResBlock (B=16, C=512, A=64, L=1024).

Data-parallel over batch: 8 cores x 2 batches each, weights replicated.
BatchNorm (training mode, stats over (B, L)) needs global batch stats ->
two tiny [128, 8] f32 AllReduces; a same-shape warmup collective at
kernel start absorbs the first-collective setup cost under input DMA.

Precision (rel-err budget 2e-2, measured ~3e-3):
  - x, xT arrive bf16 (halves input DMA); x2 and the output stay f32.
  - attnout + proj matmuls run fp8(e4m3) DoubleRow: 2 contraction
    rows/partition/cycle -> half the matmuls. xTs carries S_A=256 (the
    softmax recip is ~1e-3, below fp8 normal range), wp carries S_W=16;
    1/(S_A*S_W) folds into the x2 epilogue scalar. fp8 noise lands only
    on the attention output, which is small next to the residual.
  - convs + kq + scores stay bf16: fp8 would put ~3% relative noise on
    conv outputs (quantization noise does not average down in
    incoherent sums), blowing the budget.

Schedule:
  - Engine-order stalls sequence the DMA traffic: x/xT(b0) first, conv
    weights issue from the gpsimd queue only after the b0 queries move
    (so they never race the phase-A-critical loads; the old kernel lost
    ~10us to weight traffic queued in front of the BN1 stats DMA).
  - Phase A is software-pipelined across the two local batches: kq(b1)
    runs right after kq(b0) so b1's keys/queries are staged long before
    the PE reaches b1's scores.
  - x is pre-biased into xb = x + bp during the load window, so the x2
    epilogue is one DVE op (psum*s + xb, channel sums via accum_out);
    sums-of-squares ride ACT Square accum_out. BN stats cost no
    dedicated passes.
  - Convs run hc-outer / oc-inner with two PSUM banks (b) per step; the
    last group's epilogue is ~1.5us, and conv2 streams 512-col output
    chunks to HBM as they finish.
"""
import numpy as np

P = 128
B, C, A, L = 16, 512, 64, 1024
NCORES = 8
BL = B // NCORES          # local batches per core
CT = C // P               # 4 channel tiles
LT = L // P               # 8 length tiles
MC = L // 512             # 2 moving chunks
EPS = 1e-5
SM_SCALE = 2.0 / L        # softmax scale: scores/(L/2)
S_W = 16.0                # fp8 scale for wp
S_A = 256.0               # fp8 scale for xTs

_CACHE = {}


def _build():
    import concourse.bass as bass
    import concourse.mybir as mybir
    from concourse import bacc
    from concourse.tile import TileContext

    f32 = mybir.dt.float32
    bf16 = mybir.dt.bfloat16
    fp8 = mybir.dt.float8e4
    AF = mybir.ActivationFunctionType
    ALU = mybir.AluOpType
    DR = mybir.MatmulPerfMode.DoubleRow

    nc = bacc.Bacc(num_devices=NCORES)

    x_ext = nc.declare_dram_parameter("x", [BL, C, L], bf16, isOutput=False)
    xT_ext = nc.declare_dram_parameter("xT", [BL, L, C], bf16, isOutput=False)
    wkq_ext = nc.declare_dram_parameter("wkq", [P, CT * 2 * A], bf16, isOutput=False)
    wp_ext = nc.declare_dram_parameter("wp", [P, CT * C], fp8, isOutput=False)
    w1_ext = nc.declare_dram_parameter("w1", [P, 3 * CT * C], bf16, isOutput=False)
    w2_ext = nc.declare_dram_parameter("w2", [P, 3 * CT * C], bf16, isOutput=False)
    # per-channel vectors packed [P, CT] each: bp b1 b2 g1 be1 g2 be2, then bkq
    pvec_ext = nc.declare_dram_parameter("pvec", [P, 7 * CT + 1], f32, isOutput=False)
    out_ext = nc.declare_dram_parameter("out", [BL, C, L], f32, isOutput=True)

    cc0_in = nc.dram_tensor("cc0_in", [P, 2 * CT], f32)
    cc0_out = nc.dram_tensor("cc0_out", [P, 2 * CT], f32, addr_space="Shared")
    cc1_in = nc.dram_tensor("cc1_in", [P, 2 * CT], f32)
    cc1_out = nc.dram_tensor("cc1_out", [P, 2 * CT], f32, addr_space="Shared")
    cc2_in = nc.dram_tensor("cc2_in", [P, 2 * CT], f32)
    cc2_out = nc.dram_tensor("cc2_out", [P, 2 * CT], f32, addr_space="Shared")

    rg = [list(range(NCORES))]

    with TileContext(nc) as tc:
        with tc.tile_pool(name="pers", bufs=1) as pers, \
             tc.tile_pool(name="small", bufs=1) as small, \
             tc.tile_pool(name="ostage", bufs=4) as ostage, \
             tc.tile_pool(name="phA", bufs=2) as pab, \
             tc.tile_pool(name="psum", bufs=8, space="PSUM") as psum:

            # ---- kernel-lifetime tiles ----
            x2_sb = pers.tile([P, BL, CT, L], f32)
            x_sb = pers.tile([P, BL, CT, L], bf16)
            wkq_sb = pers.tile([P, CT, 2 * A], bf16)
            wp_sb = pers.tile([P, CT, C], fp8)
            w1_sb = pers.tile([P, 3 * CT, C], bf16)
            w2_sb = pers.tile([P, 3 * CT, C], bf16)
            h_sb = pers.tile([P, BL, CT, L + 2], bf16)
            h2_sb = pers.tile([P, BL, CT, L], bf16)
            keys_sb = pers.tile([P, BL, L], bf16)       # keys 0:A, queries A:2A
            queries_sb = pers.tile([P, BL, L], bf16)    # queries at base 0

            # warmup collective: absorbs the first-collective setup cost
            nc.gpsimd.collective_compute(
                "AllReduce", mybir.AluOpType.add, replica_groups=rg,
                ins=[cc0_in[:].opt()], outs=[cc0_out[:].opt()])

            pvec_sb = small.tile([P, 7 * CT + 1], f32, tag="pvec")
            nc.gpsimd.dma_start(out=pvec_sb[:], in_=pvec_ext[:])
            bp_sb = pvec_sb[:, 0 * CT:1 * CT]
            b1_sb = pvec_sb[:, 1 * CT:2 * CT]
            b2_sb = pvec_sb[:, 2 * CT:3 * CT]
            g1_sb = pvec_sb[:, 3 * CT:4 * CT]
            be1_sb = pvec_sb[:, 4 * CT:5 * CT]
            g2_sb = pvec_sb[:, 5 * CT:6 * CT]
            be2_sb = pvec_sb[:, 6 * CT:7 * CT]
            bkq_sb = pvec_sb[:, 7 * CT:7 * CT + 1]   # [bk; bq]

            # phase-A-critical loads on the sync queue: wkq then x
            nc.sync.dma_start(out=wkq_sb[:],
                              in_=wkq_ext[:].rearrange("p (ct a) -> p ct a", ct=CT))
            for b in range(BL):
                for mc in range(MC):
                    for ct in range(CT):
                        nc.sync.dma_start(
                            out=x_sb[:, b, ct, mc * 512:(mc + 1) * 512],
                            in_=x_ext[b, ct * P:(ct + 1) * P, mc * 512:(mc + 1) * 512])
            # xT(b0) early on gpsimd; xT(b1) + conv weights issue later from
            # the same queue, after it stalls on the b0 queries move
            xT_tiles = [pab.tile([P, LT, C], bf16, tag="xT", name=f"xT{_b}")
                        for _b in range(BL)]
            nc.gpsimd.dma_start(out=xT_tiles[0][:],
                                in_=xT_ext[0].rearrange("(lc p) c -> p lc c", p=P))

            ccin1_sb = small.tile([P, 2 * CT], f32, tag="ccin1")
            ccout1_sb = small.tile([P, 2 * CT], f32, tag="ccout1")
            ccin2_sb = small.tile([P, 2 * CT], f32, tag="ccin2")
            ccout2_sb = small.tile([P, 2 * CT], f32, tag="ccout2")
            # per-chunk stat accumulators [P, ct, 2*b+chunk]
            m1a = small.tile([P, CT, 2 * BL], f32, tag="m1a")   # sum(x2)
            m2a = small.tile([P, CT, 2 * BL], f32, tag="m2a")   # sum(x2^2)
            n1a = small.tile([P, CT, 2 * BL], f32, tag="n1a")   # sum(h2)
            n2a = small.tile([P, CT, 2 * BL], f32, tag="n2a")   # sum(h2^2)
            scale1 = small.tile([P, CT], f32, tag="scale1")
            bias1 = small.tile([P, CT], f32, tag="bias1")
            scale2 = small.tile([P, CT], f32, tag="scale2")
            bias2 = small.tile([P, CT], f32, tag="bias2")
            eps_sb = small.tile([P, 1], f32, tag="eps")
            nc.vector.memset(eps_sb[:], EPS)

            # conv padding zeros (columns 0 and L+1 of every (b, ct) row)
            nc.vector.memset(h_sb[:, :, :, 0], 0.0)
            nc.vector.memset(h_sb[:, :, :, L + 1], 0.0)

            # pre-warm ACT function tables (only 4 funcs used all kernel)
            warm = small.tile([P, 1], f32, tag="warm")
            for fn in (AF.Square, AF.Sqrt, AF.Relu, AF.Exp):
                nc.scalar.activation(out=warm[:], in_=eps_sb[:], func=fn)

            def stats_allreduce(ccin_dram, ccout_dram, ccin_sb, ccred_sb):
                nc.gpsimd.dma_start(out=ccin_dram[:], in_=ccin_sb[:])
                nc.gpsimd.collective_compute(
                    "AllReduce", mybir.AluOpType.add, replica_groups=rg,
                    ins=[ccin_dram[:].opt()], outs=[ccout_dram[:].opt()])
                nc.gpsimd.dma_start(out=ccred_sb[:], in_=ccout_dram[:])

            def pack_stats(msum, sqsum, ccin_sb):
                nc.vector.tensor_reduce(out=ccin_sb[:, 0:CT], in_=msum[:],
                                        axis=mybir.AxisListType.X, op=ALU.add)
                nc.vector.tensor_reduce(out=ccin_sb[:, CT:2 * CT], in_=sqsum[:],
                                        axis=mybir.AxisListType.X, op=ALU.add)

            def bn_post(ccout_sb, g_sb, be_sb, scale_t, bias_t, tag):
                mgx = small.tile([P, 2 * CT], f32, tag=tag + "mgx")
                nc.vector.tensor_scalar_mul(out=mgx[:], in0=ccout_sb[:],
                                            scalar1=1.0 / (B * L))
                mg = mgx[:, 0:CT]
                ex2 = mgx[:, CT:2 * CT]
                nvar = small.tile([P, CT], f32, tag=tag + "nv")
                # nvar = mean^2 - E[x^2] = -var
                nc.vector.tensor_tensor(out=nvar[:], in0=mg, in1=mg, op=ALU.mult)
                nc.vector.tensor_tensor(out=nvar[:], in0=nvar[:], in1=ex2, op=ALU.subtract)
                sd = small.tile([P, CT], f32, tag=tag + "sd")
                nc.scalar.activation(out=sd[:], in_=nvar[:], func=AF.Sqrt,
                                     scale=-1.0, bias=eps_sb[:])
                rstd = small.tile([P, CT], f32, tag=tag + "rstd")
                nc.vector.reciprocal(out=rstd[:], in_=sd[:])
                nc.vector.tensor_tensor(out=scale_t[:], in0=rstd[:], in1=g_sb[:], op=ALU.mult)
                tmp = small.tile([P, CT], f32, tag=tag + "tmp")
                nc.vector.tensor_tensor(out=tmp[:], in0=mg, in1=scale_t[:], op=ALU.mult)
                nc.vector.tensor_tensor(out=bias_t[:], in0=be_sb[:], in1=tmp[:], op=ALU.subtract)

            # ---------------- Phase A: attention ----------------
            # kq for BOTH batches first (pipelines b1's staging under b0)
            for b in range(BL):
                for mc in range(MC):
                    ms = slice(mc * 512, (mc + 1) * 512)
                    kps = psum.tile([P, 512], f32, tag="ps")
                    for ct in range(CT):
                        nc.tensor.matmul(
                            out=kps[:],
                            lhsT=wkq_sb[:, ct, :],
                            rhs=x_sb[:, b, ct, ms],
                            start=(ct == 0), stop=(ct == CT - 1))
                    # rows 0:64 keys+bk, 64:128 queries+bq (one DVE op)
                    nc.vector.tensor_scalar_add(out=keys_sb[:, b, ms],
                                                in0=kps[:], scalar1=bkq_sb)
                # queries to partition base 0 (SBUF->SBUF remap)
                nc.gpsimd.dma_start(out=queries_sb[0:A, b, :],
                                    in_=keys_sb[A:2 * A, b, :])
                if b == 0:
                    # gpsimd reaches here only after the b0 queries move:
                    # xT(b1) + conv weights now load behind phase-A traffic
                    nc.gpsimd.dma_start(out=xT_tiles[1][:],
                                        in_=xT_ext[1].rearrange("(lc p) c -> p lc c", p=P))
                    nc.gpsimd.dma_start(out=w1_sb[:],
                                        in_=w1_ext[:].rearrange("p (kc c) -> p kc c", c=C))
                    nc.gpsimd.dma_start(out=w2_sb[:],
                                        in_=w2_ext[:].rearrange("p (kc c) -> p kc c", c=C))
                    nc.gpsimd.dma_start(out=wp_sb[:],
                                        in_=wp_ext[:].rearrange("p (ct o) -> p ct o", ct=CT))

            for b in range(BL):
                xT_sb = xT_tiles[b]
                e_sb = pab.tile([P, LT, L], fp8, tag="e")
                rsp = pab.tile([P, LT, MC], f32, tag="rsp")
                rcp = pab.tile([P, LT], f32, tag="rcp")
                xTs = pab.tile([P, LT, C], fp8, tag="xTs")
                for lc in range(LT):
                    for mc in range(MC):
                        sps = psum.tile([P, 512], f32, tag="ps")
                        nc.tensor.matmul(
                            out=sps[:],
                            lhsT=keys_sb[0:A, b, lc * P:(lc + 1) * P],
                            rhs=queries_sb[0:A, b, mc * 512:(mc + 1) * 512],
                            start=True, stop=True)
                        # exp -> fp8, free row sums via the ACT accumulator
                        nc.scalar.activation(
                            out=e_sb[:, lc, mc * 512:(mc + 1) * 512],
                            in_=sps[:], func=AF.Exp, scale=SM_SCALE,
                            accum_out=rsp[:, lc, mc:mc + 1])
                    nc.vector.scalar_tensor_tensor(
                        out=rcp[:, lc:lc + 1], in0=rsp[:, lc, 0:1],
                        scalar=1.0, in1=rsp[:, lc, 1:2],
                        op0=ALU.mult, op1=ALU.add)
                    nc.vector.reciprocal(out=rcp[:, lc:lc + 1],
                                         in_=rcp[:, lc:lc + 1])
                    # xTs[l, c] = xT[l, c] * (S_A / rowsum[l]) -> fp8
                    nc.vector.tensor_scalar(out=xTs[:, lc, :],
                                            in0=xT_sb[:, lc, :],
                                            scalar1=rcp[:, lc:lc + 1],
                                            scalar2=S_A,
                                            op0=ALU.mult, op1=ALU.mult)

                ao_sb = pab.tile([P, CT, L], fp8, tag="ao")
                for cc in range(CT):
                    for mc in range(MC):
                        ms = slice(mc * 512, (mc + 1) * 512)
                        aps = psum.tile([P, 512], f32, tag="ps")
                        for lcp in range(LT // 2):
                            nc.tensor.matmul(
                                out=aps[:],
                                lhsT=xTs[:, 2 * lcp:2 * lcp + 2, cc * P:(cc + 1) * P],
                                rhs=e_sb[:, 2 * lcp:2 * lcp + 2, ms],
                                start=(lcp == 0), stop=(lcp == LT // 2 - 1),
                                perf_mode=DR)
                        nc.scalar.activation(out=ao_sb[:, cc, ms],
                                             in_=aps[:], func=AF.Copy)

                for oc in range(CT):
                    for mc in range(MC):
                        ms = slice(mc * 512, (mc + 1) * 512)
                        pps = psum.tile([P, 512], f32, tag="ps")
                        for cp in range(CT // 2):
                            nc.tensor.matmul(
                                out=pps[:],
                                lhsT=wp_sb[:, 2 * cp:2 * cp + 2, oc * P:(oc + 1) * P],
                                rhs=ao_sb[:, 2 * cp:2 * cp + 2, ms],
                                start=(cp == 0), stop=(cp == CT // 2 - 1),
                                perf_mode=DR)
                        # x2' = psum/(S_W*S_A) + x  (bp deferred: BN1 is
                        # invariant to per-channel constants; bp rejoins in
                        # the conv2 epilogue via b2+bp)
                        nc.vector.scalar_tensor_tensor(
                            out=x2_sb[:, b, oc, ms], in0=pps[:],
                            scalar=1.0 / (S_W * S_A), in1=x_sb[:, b, oc, ms],
                            op0=ALU.mult, op1=ALU.add,
                            accum_out=m1a[:, oc, 2 * b + mc:2 * b + mc + 1])
                        sqs = ostage.tile([P, 512], f32, tag="sqs")
                        nc.vector.scalar_tensor_tensor(
                            out=sqs[:], in0=x2_sb[:, b, oc, ms],
                            scalar=1.0, in1=x2_sb[:, b, oc, ms],
                            op0=ALU.mult, op1=ALU.mult,
                            accum_out=m2a[:, oc, 2 * b + mc:2 * b + mc + 1])

            # ---------------- BN1 + conv1 ----------------
            pack_stats(m1a, m2a, ccin1_sb)
            stats_allreduce(cc1_in, cc1_out, ccin1_sb, ccout1_sb)
            for fn in (AF.Sqrt, AF.Relu):   # re-warm while the mesh runs
                nc.scalar.activation(out=warm[:], in_=eps_sb[:], func=fn)
            bn_post(ccout1_sb, g1_sb, be1_sb, scale1, bias1, "p1")

            # h = relu(bn1(x2)), padded; b-major so conv can start after 2 ops
            for b in range(BL):
                for ct in range(CT):
                    nc.scalar.activation(out=h_sb[:, b, ct, 1:L + 1],
                                         in_=x2_sb[:, b, ct, :], func=AF.Relu,
                                         scale=scale1[:, ct:ct + 1],
                                         bias=bias1[:, ct:ct + 1])

            # conv1: hc-outer, oc-inner, 2 psum banks (b)
            for hc in range(MC):
                for oc in range(CT):
                    cps = [psum.tile([P, 512], f32, tag="ps", name=f"c1ps{hc}_{oc}_{_j}")
                           for _j in range(BL)]
                    for ct in range(CT):
                        for k in range(3):
                            w_ap = w1_sb[:, k * CT + ct, oc * P:(oc + 1) * P]
                            for b in range(BL):
                                nc.tensor.matmul(
                                    out=cps[b][:], lhsT=w_ap,
                                    rhs=h_sb[:, b, ct, hc * 512 + k:hc * 512 + k + 512],
                                    start=(ct == 0 and k == 0),
                                    stop=(ct == CT - 1 and k == 2))
                    for b in range(BL):
                        hs = slice(hc * 512, (hc + 1) * 512)
                        nc.vector.tensor_scalar(
                            out=h2_sb[:, b, oc, hs], in0=cps[b][:],
                            scalar1=b1_sb[:, oc:oc + 1], scalar2=0.0,
                            op0=ALU.add, op1=ALU.add,
                            accum_out=n1a[:, oc, 2 * b + hc:2 * b + hc + 1])
                        sqs = ostage.tile([P, 512], f32, tag="sqs")
                        nc.scalar.activation(
                            out=sqs[:], in_=h2_sb[:, b, oc, hs], func=AF.Square,
                            accum_out=n2a[:, oc, 2 * b + hc:2 * b + hc + 1])

            # ---------------- BN2 + conv2 ----------------
            pack_stats(n1a, n2a, ccin2_sb)
            stats_allreduce(cc2_in, cc2_out, ccin2_sb, ccout2_sb)
            for fn in (AF.Sqrt, AF.Relu):   # re-warm while the mesh runs
                nc.scalar.activation(out=warm[:], in_=eps_sb[:], func=fn)
            bn_post(ccout2_sb, g2_sb, be2_sb, scale2, bias2, "p2")

            # h3 = relu(bn2(h2)) overwrites h_sb in place (pad zeros kept)
            for b in range(BL):
                for ct in range(CT):
                    nc.scalar.activation(out=h_sb[:, b, ct, 1:L + 1],
                                         in_=h2_sb[:, b, ct, :], func=AF.Relu,
                                         scale=scale2[:, ct:ct + 1],
                                         bias=bias2[:, ct:ct + 1])

            # conv2 + b2 + residual -> out, streaming 512-col chunks to HBM
            for hc in range(MC):
                for oc in range(CT):
                    cps = [psum.tile([P, 512], f32, tag="ps", name=f"c2ps{hc}_{oc}_{_j}")
                           for _j in range(BL)]
                    for ct in range(CT):
                        for k in range(3):
                            w_ap = w2_sb[:, k * CT + ct, oc * P:(oc + 1) * P]
                            for b in range(BL):
                                nc.tensor.matmul(
                                    out=cps[b][:], lhsT=w_ap,
                                    rhs=h_sb[:, b, ct, hc * 512 + k:hc * 512 + k + 512],
                                    start=(ct == 0 and k == 0),
                                    stop=(ct == CT - 1 and k == 2))
                    for b in range(BL):
                        hs = slice(hc * 512, (hc + 1) * 512)
                        og = ostage.tile([P, 512], f32, tag="og")
                        nc.vector.scalar_tensor_tensor(
                            out=og[:], in0=cps[b][:],
                            scalar=b2_sb[:, oc:oc + 1],
                            in1=x2_sb[:, b, oc, hs],
                            op0=ALU.add, op1=ALU.add)
                        nc.sync.dma_start(
                            out=out_ext[b, oc * P:(oc + 1) * P, hs], in_=og[:])

    nc.compile()
    return nc


def _get_nc():
    if "nc" not in _CACHE:
        _CACHE["nc"] = _build()
    return _CACHE["nc"]


def _prep_in_maps(inputs):
    import ml_dtypes
    f = np.float32
    bf = ml_dtypes.bfloat16
    f8 = ml_dtypes.float8_e4m3
    x = np.ascontiguousarray(inputs["x"], dtype=f)

    def vec_pct(v):
        # (C,) -> [P, CT] with channel c = ct*P + p at [p, ct]
        return np.asarray(v, dtype=f).reshape(CT, P).T

    pvec = np.concatenate(
        [vec_pct(inputs["bp"]), vec_pct(inputs["b1"]),
         vec_pct(inputs["b2"]) + vec_pct(inputs["bp"]),
         vec_pct(inputs["g1"]), vec_pct(inputs["be1"]),
         vec_pct(inputs["g2"]), vec_pct(inputs["be2"]),
         np.concatenate([inputs["bk"], inputs["bq"]]).reshape(P, 1).astype(f)],
        axis=1)

    def swiz2(w):  # [C, X] -> [P, CT*X] partition-major
        X = w.shape[1]
        return np.ascontiguousarray(
            w.reshape(CT, P, X).transpose(1, 0, 2).reshape(P, CT * X))

    def swiz3(w):  # [3, C, C] (k, i, o) -> [P, 3*CT*C] with cols (k*CT+ct)*C+o
        return np.ascontiguousarray(
            w.reshape(3, CT, P, C).transpose(2, 0, 1, 3).reshape(P, 3 * CT * C))

    shared = {
        "wkq": swiz2(np.concatenate([inputs["Wk"].T, inputs["Wq"].T], axis=1).astype(bf)),
        "wp": swiz2((inputs["Wp"].T * S_W).astype(f8)),
        "w1": swiz3(np.transpose(inputs["W1"], (2, 1, 0)).astype(bf)),
        "w2": swiz3(np.transpose(inputs["W2"], (2, 1, 0)).astype(bf)),
        "pvec": np.ascontiguousarray(pvec, dtype=f),
    }
    in_maps = []
    for i in range(NCORES):
        xl = np.ascontiguousarray(x[i * BL:(i + 1) * BL])
        xTl = np.ascontiguousarray(np.transpose(xl, (0, 2, 1)).astype(bf))
        m = {"x": xl.astype(bf), "xT": xTl}
        m.update(shared)
        in_maps.append(m)
    return in_maps


def kernel(**inputs) -> np.ndarray:
    from concourse import bass_utils
    nc = _get_nc()
    in_maps = _prep_in_maps(inputs)
    res = bass_utils.run_bass_kernel_spmd(nc, in_maps, list(range(NCORES)))
    return np.concatenate([r["out"] for r in res.results], axis=0)
